# revision 20
# baseline (speedup 1.0000x reference)
"""Causal attention block (q,k,v,mask -> (output, attention)) on 8 trn2 cores.

Sharding: B*H = 32 head-slices split 4-per-core across 8 cores (head
parallel, zero communication). Each core computes, for its 4 heads:
    scores = (q @ k^T) / sqrt(d), masked additively, softmax, out = A @ v
and writes both the [S,S] attention matrix and the [S,D] output.

Key structure per (head, 128-row q-block):
  - PE: scores chunk matmuls (QT stationary), penalty added into PSUM via
    an identity-weighted matmul (I.T @ pen == pen, accumulated).
  - ACT: exp straight from PSUM with scale=1/sqrt(d) and fused row-sum
    (accum_out) -> unnormalized E in the SBUF quarter buffer.
  - DVE: reciprocal of row sums, then in-place normalize (2x perf mode).
  - PE: transpose normalized A tiles (128x128) -> PSUM, evacuated into a
    [k, q] staging buffer for the second matmul.
  - PE: out^T = sum_k V[k]^T-stationary matmuls over A^T, then transposed
    back and written out.
Masked upper-triangle attention entries are exactly 0 in the reference
(exp underflow) and the runtime pre-zeroes output buffers, so the causal
path only writes the lower-triangle staircase.
"""

import math
import os

import numpy as np

import concourse.bass as bass
import concourse.tile as tile
from concourse import bacc, mybir
from concourse.bass_utils import run_bass_kernel_spmd

FP32 = mybir.dt.float32
F16 = mybir.dt.float16
AF = mybir.ActivationFunctionType

B, H, S, D = 2, 16, 2048, 64
N_CORES = 8
HEADS_PER_CORE = (B * H) // N_CORES  # 4
NB = S // 128  # 16 q/k blocks per head
NQUARTERS = 4
QBLKS = NB // NQUARTERS  # 4 q-blocks per quarter

# Penalty in raw (pre-scale) score space; exp(scale * -8e9) underflows to
# exactly 0.0f, matching the reference's masked_fill(-1e9) -> softmax -> 0.
PENALTY = -8.0e9


def _attention_body(tc, outs, ins, causal: bool, scale: float):
    nc = tc.nc
    q_d, k_d, v_d = ins["q"], ins["k"], ins["v"]
    pen_d, id_d = ins["pen"], ins["ident"]
    att_d, out_d = outs["att"], outs["out"]

    with (
        tc.tile_pool(name="singles", bufs=1) as singles,
        tc.tile_pool(name="qkv", bufs=2) as qkv_pool,
        tc.tile_pool(name="qt", bufs=2) as qt_pool,
        tc.tile_pool(name="aq", bufs=2) as aq_pool,
        tc.tile_pool(name="a16", bufs=3) as a16_pool,
        tc.tile_pool(name="at", bufs=1) as at_pool,
        tc.tile_pool(name="pent", bufs=2) as pen_pool,
        tc.tile_pool(name="sums", bufs=8) as sums_pool,
        tc.tile_pool(name="o", bufs=2) as o_pool,
        tc.tile_pool(name="ps_scores", bufs=2, space="PSUM") as ps_scores,
        tc.tile_pool(name="ps_at", bufs=2, space="PSUM") as ps_at,
        tc.tile_pool(name="ps_ot", bufs=1, space="PSUM") as ps_ot,
        tc.tile_pool(name="ps_misc", bufs=1, space="PSUM") as ps_misc,
    ):
        ident = singles.tile([128, 128], FP32)
        nc.gpsimd.dma_start(out=ident, in_=id_d)
        ident16 = singles.tile([128, 128], F16)
        nc.gpsimd.tensor_copy(ident16, ident)
        if causal:
            # one [128,128] penalty block per q-block index, side by side
            pen = singles.tile([128, S], FP32)
            nc.gpsimd.dma_start(out=pen, in_=pen_d)

        for h in range(HEADS_PER_CORE):
            # ---- load this head's q, k, v as 16 x [128, 64] chunks ----
            Qn = qkv_pool.tile([128, NB, 64], FP32, tag="Qn")
            Kn = qkv_pool.tile([128, NB, 64], FP32, tag="Kn")
            V16 = qkv_pool.tile([128, NB, 64], F16, tag="V16")
            nc.gpsimd.dma_start(
                out=Qn, in_=q_d[h].rearrange("(c p) d -> p c d", p=128)
            )
            nc.gpsimd.dma_start(
                out=Kn, in_=k_d[h].rearrange("(c p) d -> p c d", p=128)
            )
            # cast to bf16 during the DMA (SWDGE): v only feeds the A@V
            # matmul, whose precision is dominated by the bf16 A^T anyway
            nc.gpsimd.dma_start(
                out=V16, in_=v_d[h].rearrange("(c p) d -> p c d", p=128)
            )

            # ---- build QT, KT = [64, S] transposed layouts via PE ----
            QT = qt_pool.tile([64, S], FP32, tag="QT")
            KT = qt_pool.tile([64, S], FP32, tag="KT")
            for src, dst, use_act in ((Qn, QT, True), (Kn, KT, False)):
                for g in range(NB // 4):
                    pt = ps_misc.tile([64, 512], FP32, tag="misc")
                    for j in range(4):
                        c = g * 4 + j
                        # exact fp32 transpose as a regular matmul: src.T @ I
                        nc.tensor.matmul(
                            pt[:, j * 128 : (j + 1) * 128],
                            lhsT=src[:, c, :],
                            rhs=ident,
                            start=True,
                            stop=True,
                        )
                    if use_act:
                        nc.scalar.copy(dst[:, g * 512 : (g + 1) * 512], pt)
                    else:
                        nc.vector.tensor_copy(dst[:, g * 512 : (g + 1) * 512], pt)

            for qb in range(NQUARTERS):
                kvmax = (qb * QBLKS + QBLKS) * 128 if causal else S
                kcmax = kvmax // 128  # k-chunks consumed by this quarter
                # A quarter buffer: 4 q-blocks x full key row
                Aq = aq_pool.tile([128, QBLKS, S], FP32, tag="Aq")
                # A^T staging: [k-part, k-chunk, q-within-quarter]
                AT = at_pool.tile([128, NB, 512], F16, tag="AT")

                for i in range(QBLKS):
                    qi = qb * QBLKS + i
                    kv = (qi + 1) * 128 if causal else S

                    if not causal:
                        pent = pen_pool.tile([128, S], FP32, tag="pent")
                        nc.gpsimd.dma_start(
                            out=pent, in_=pen_d[qi * 128 : (qi + 1) * 128, :]
                        )

                    # ---- scores into PSUM, [128,1024] tiles ----
                    stiles = []
                    for c0 in range(0, kv, 1024):
                        w = min(1024, kv - c0)
                        st = ps_scores.tile([128, 1024], FP32, tag="scores")
                        stiles.append((st, c0, w))
                        for n0 in range(0, w, 512):
                            n = min(512, w - n0)
                            nc.tensor.matmul(
                                st[:, n0 : n0 + n],
                                lhsT=QT[:, qi * 128 : (qi + 1) * 128],
                                rhs=KT[:, c0 + n0 : c0 + n0 + n],
                                start=True,
                                stop=True,
                            )
                        if not causal:
                            # add penalty rows into every chunk: I.T @ pen
                            for n0 in range(0, w, 512):
                                n = min(512, w - n0)
                                nc.tensor.matmul(
                                    st[:, n0 : n0 + n],
                                    lhsT=ident,
                                    rhs=pent[:, c0 + n0 : c0 + n0 + n],
                                    start=False,
                                    stop=True,
                                    skip_group_check=True,
                                )
                    if causal:
                        # only the diagonal block needs masking (I.T @ pen
                        # accumulated into PSUM; in-place DVE adds on PSUM
                        # are not safe - single-ported banks)
                        st, c0, _w = stiles[-1]
                        off = qi * 128 - c0
                        nc.tensor.matmul(
                            st[:, off : off + 128],
                            lhsT=ident,
                            rhs=pen[:, qi * 128 : (qi + 1) * 128],
                            start=False,
                            stop=True,
                            skip_group_check=True,
                        )

                    # ---- exp from PSUM with fused row sums ----
                    sums_parts = sums_pool.tile([128, 2], FP32, tag="sums")
                    for t, (st, c0, w) in enumerate(stiles):
                        nc.scalar.activation(
                            out=Aq[:, i, c0 : c0 + w],
                            in_=st[:, 0:w],
                            func=AF.Exp,
                            scale=scale,
                            accum_out=sums_parts[:, t : t + 1],
                        )

                    recip = sums_pool.tile([128, 1], FP32, tag="recip")
                    if len(stiles) > 1:
                        ssum = sums_pool.tile([128, 1], FP32, tag="ssum")
                        nc.vector.tensor_add(
                            ssum, sums_parts[:, 0:1], sums_parts[:, 1:2]
                        )
                        nc.vector.reciprocal(recip, ssum)
                    else:
                        nc.vector.reciprocal(recip, sums_parts[:, 0:1])

                    # ---- normalize in place (DVE 2x) ----
                    nc.vector.tensor_scalar_mul(
                        Aq[:, i, 0:kv], Aq[:, i, 0:kv], recip
                    )

                    # ---- bf16 copy of A for the A@V path (gpsimd, idle) ----
                    A16 = a16_pool.tile([128, S], F16, tag="A16")
                    nc.gpsimd.tensor_copy(A16[:, 0:kv], Aq[:, i, 0:kv])

                    # ---- transpose A row-block into AT staging (bf16) ----
                    nkc = kv // 128
                    for g0 in range(0, nkc, 4):
                        gn = min(4, nkc - g0)
                        pat = ps_at.tile([128, 512], FP32, tag="pat")
                        for j in range(gn):
                            kc = g0 + j
                            nc.tensor.matmul(
                                pat[:, j * 128 : (j + 1) * 128],
                                lhsT=A16[:, kc * 128 : (kc + 1) * 128],
                                rhs=ident16,
                                start=True,
                                stop=True,
                            )
                        dst = AT[:, g0 : g0 + gn, i * 128 : (i + 1) * 128]
                        src = pat[:, 0 : gn * 128].rearrange(
                            "p (g j) -> p g j", g=gn
                        )
                        if (g0 // 4) % 2 == 0:
                            nc.vector.tensor_copy(dst, src)
                        else:
                            nc.scalar.copy(dst, src)
                    if causal:
                        # zero the not-yet-attended staircase inside quarter
                        for kc in range(nkc, kcmax):
                            nc.gpsimd.memset(
                                AT[:, kc, i * 128 : (i + 1) * 128], 0.0
                            )

                if causal:
                    # zero Aq staircase so one rectangular DMA covers quarter
                    for i in range(QBLKS - 1):
                        kv_i = (qb * QBLKS + i + 1) * 128
                        nc.gpsimd.memset(Aq[:, i, kv_i:kvmax], 0.0)

                # ---- attention quarter write ----
                att_dst = att_d[h, qb * 512 : (qb + 1) * 512, 0:kvmax].rearrange(
                    "(c p) k -> p c k", p=128
                )
                nc.sync.dma_start(out=att_dst, in_=Aq[:, :, 0:kvmax])

                # ---- out^T = sum_kc V[kc]-stationary @ A^T[kc] ----
                poT = ps_ot.tile([64, 512], FP32, tag="poT")
                for kc in range(kcmax):
                    nc.tensor.matmul(
                        poT,
                        lhsT=V16[:, kc, :],
                        rhs=AT[:, kc, :],
                        start=(kc == 0),
                        stop=(kc == kcmax - 1),
                    )
                oT = o_pool.tile([64, 512], FP32, tag="oT")
                nc.scalar.copy(oT, poT)
                pout = ps_misc.tile([128, 256], FP32, tag="misc")
                for j in range(4):
                    nc.tensor.matmul(
                        pout[:, j * 64 : (j + 1) * 64],
                        lhsT=oT[:, j * 128 : (j + 1) * 128],
                        rhs=ident[0:64, 0:64],
                        start=True,
                        stop=True,
                    )
                ob = o_pool.tile([128, 4, 64], FP32, tag="ob")
                nc.vector.tensor_copy(
                    ob, pout.rearrange("p (c d) -> p c d", c=4)
                )
                out_dst = out_d[h, qb * 512 : (qb + 1) * 512, :].rearrange(
                    "(c p) d -> p c d", p=128
                )
                nc.sync.dma_start(out=out_dst, in_=ob)


def build_program(causal: bool, scale: float):
    nc = bacc.Bacc(
        "TRN2",
        target_bir_lowering=False,
        debug=False,
        enable_asserts=False,
        num_devices=N_CORES,
    )
    hp = HEADS_PER_CORE
    ins = {
        "q": nc.dram_tensor("q", [hp, S, D], FP32, kind="ExternalInput").ap(),
        "k": nc.dram_tensor("k", [hp, S, D], FP32, kind="ExternalInput").ap(),
        "v": nc.dram_tensor("v", [hp, S, D], FP32, kind="ExternalInput").ap(),
        "ident": nc.dram_tensor(
            "ident", [128, 128], FP32, kind="ExternalInput"
        ).ap(),
    }
    pen_shape = [128, S] if causal else [S, S]
    ins["pen"] = nc.dram_tensor(
        "pen", pen_shape, FP32, kind="ExternalInput"
    ).ap()
    outs = {
        "att": nc.dram_tensor(
            "att", [hp, S, S], FP32, kind="ExternalOutput"
        ).ap(),
        "out": nc.dram_tensor(
            "out", [hp, S, D], FP32, kind="ExternalOutput"
        ).ap(),
    }
    with tile.TileContext(nc) as tc:
        _attention_body(tc, outs, ins, causal=causal, scale=scale)
    nc.compile()
    return nc


_PROGRAM_CACHE = {}


def _get_program(causal: bool, scale: float):
    key = (causal, scale)
    if key not in _PROGRAM_CACHE:
        _PROGRAM_CACHE[key] = build_program(causal, scale)
    return _PROGRAM_CACHE[key]


def kernel(q, k, v, mask, d_key, mask_value):
    q = np.ascontiguousarray(np.asarray(q, dtype=np.float32))
    k = np.ascontiguousarray(np.asarray(k, dtype=np.float32))
    v = np.ascontiguousarray(np.asarray(v, dtype=np.float32))
    mask2d = np.asarray(mask).reshape(S, S)
    scale = 1.0 / math.sqrt(float(np.asarray(d_key)))

    causal = bool(
        np.array_equal(mask2d != 0, np.tril(np.ones((S, S), dtype=bool)))
    )

    maskf = (mask2d != 0).astype(np.float32)
    if causal:
        # per-q-block diagonal penalty blocks, stacked along free dim
        pen = np.zeros((128, S), dtype=np.float32)
        for qi in range(NB):
            blk = maskf[qi * 128 : (qi + 1) * 128, qi * 128 : (qi + 1) * 128]
            pen[:, qi * 128 : (qi + 1) * 128] = (1.0 - blk) * PENALTY
    else:
        pen = (1.0 - maskf) * PENALTY

    ident = np.eye(128, dtype=np.float32)

    qr = q.reshape(B * H, S, D)
    kr = k.reshape(B * H, S, D)
    vr = v.reshape(B * H, S, D)

    nc = _get_program(causal, scale)
    in_maps = []
    for c in range(N_CORES):
        sl = slice(c * HEADS_PER_CORE, (c + 1) * HEADS_PER_CORE)
        in_maps.append(
            {
                "q": np.ascontiguousarray(qr[sl]),
                "k": np.ascontiguousarray(kr[sl]),
                "v": np.ascontiguousarray(vr[sl]),
                "pen": pen,
                "ident": ident,
            }
        )

    trace = os.environ.get("KERNEL_TRACE") == "1"
    res = run_bass_kernel_spmd(
        nc, in_maps, core_ids=list(range(N_CORES)), trace=trace
    )
    if trace:
        print(f"HW exec time: {res.exec_time_ns} ns")

    att = np.empty((B * H, S, S), dtype=np.float32)
    out = np.empty((B * H, S, D), dtype=np.float32)
    for c in range(N_CORES):
        sl = slice(c * HEADS_PER_CORE, (c + 1) * HEADS_PER_CORE)
        att[sl] = res.results[c]["att"]
        out[sl] = res.results[c]["out"]
    return out.reshape(B, H, S, D), att.reshape(B, H, S, S)


# revision 21
# speedup vs baseline: 1.1574x; 1.1574x over previous
"""Causal attention block (q,k,v,mask -> (output, attention)) on 8 trn2 cores.

Sharding: B*H = 32 head-slices split 4-per-core across 8 cores (head
parallel, zero communication). Each core computes, for its 4 heads:
    scores = (q @ k^T) / sqrt(d), masked additively, softmax, out = A @ v
and writes both the [S,S] attention matrix and the [S,D] output.

Key structure per (head, 128-row q-block):
  - PE: scores chunk matmuls (QT stationary), penalty added into PSUM via
    an identity-weighted matmul (I.T @ pen == pen, accumulated).
  - ACT: exp straight from PSUM with scale=1/sqrt(d) and fused row-sum
    (accum_out) -> unnormalized E in the SBUF quarter buffer.
  - DVE: reciprocal of row sums, then in-place normalize (2x perf mode).
  - PE: transpose normalized A tiles (128x128) -> PSUM, evacuated into a
    [k, q] staging buffer for the second matmul.
  - PE: out^T = sum_k V[k]^T-stationary matmuls over A^T, then transposed
    back and written out.
Masked upper-triangle attention entries are exactly 0 in the reference
(exp underflow) and the runtime pre-zeroes output buffers, so the causal
path only writes the lower-triangle staircase.
"""

import math
import os

import numpy as np

import concourse.bass as bass
import concourse.tile as tile
from concourse import bacc, mybir
from concourse.bass_utils import run_bass_kernel_spmd

FP32 = mybir.dt.float32
F16 = mybir.dt.float16
AF = mybir.ActivationFunctionType

B, H, S, D = 2, 16, 2048, 64
N_CORES = 8
HEADS_PER_CORE = (B * H) // N_CORES  # 4
NB = S // 128  # 16 q/k blocks per head
NQUARTERS = 4
QBLKS = NB // NQUARTERS  # 4 q-blocks per quarter

# Penalty in raw (pre-scale) score space; exp(scale * -8e9) underflows to
# exactly 0.0f, matching the reference's masked_fill(-1e9) -> softmax -> 0.
PENALTY = -8.0e9


def _attention_body(tc, outs, ins, causal: bool, scale: float):
    nc = tc.nc
    q_d, k_d, v_d = ins["q"], ins["k"], ins["v"]
    pen_d, id_d = ins["pen"], ins["ident"]
    att_d, out_d = outs["att"], outs["out"]

    with (
        tc.tile_pool(name="singles", bufs=1) as singles,
        tc.tile_pool(name="qkv", bufs=2) as qkv_pool,
        tc.tile_pool(name="qt", bufs=2) as qt_pool,
        tc.tile_pool(name="aq", bufs=2) as aq_pool,
        tc.tile_pool(name="a16", bufs=3) as a16_pool,
        tc.tile_pool(name="at", bufs=1) as at_pool,
        tc.tile_pool(name="pent", bufs=2) as pen_pool,
        tc.tile_pool(name="sums", bufs=8) as sums_pool,
        tc.tile_pool(name="o", bufs=2) as o_pool,
        tc.tile_pool(name="ps_scores", bufs=2, space="PSUM") as ps_scores,
        tc.tile_pool(name="ps_at", bufs=2, space="PSUM") as ps_at,
        tc.tile_pool(name="ps_ot", bufs=1, space="PSUM") as ps_ot,
        tc.tile_pool(name="ps_misc", bufs=1, space="PSUM") as ps_misc,
    ):
        ident = singles.tile([128, 128], FP32)
        nc.gpsimd.dma_start(out=ident, in_=id_d)
        ident16 = singles.tile([128, 128], F16)
        nc.gpsimd.tensor_copy(ident16, ident)
        if causal:
            # one [128,128] penalty block per q-block index, side by side
            pen = singles.tile([128, S], FP32)
            nc.gpsimd.dma_start(out=pen, in_=pen_d)

        for h in range(HEADS_PER_CORE):
            # ---- load this head's q, k, v as 16 x [128, 64] chunks ----
            Qn = qkv_pool.tile([128, NB, 64], FP32, tag="Qn")
            Kn = qkv_pool.tile([128, NB, 64], FP32, tag="Kn")
            V16 = qkv_pool.tile([128, NB, 64], F16, tag="V16")
            nc.gpsimd.dma_start(
                out=Qn, in_=q_d[h].rearrange("(c p) d -> p c d", p=128)
            )
            nc.gpsimd.dma_start(
                out=Kn, in_=k_d[h].rearrange("(c p) d -> p c d", p=128)
            )
            # cast to bf16 during the DMA (SWDGE): v only feeds the A@V
            # matmul, whose precision is dominated by the bf16 A^T anyway
            nc.gpsimd.dma_start(
                out=V16, in_=v_d[h].rearrange("(c p) d -> p c d", p=128)
            )

            # ---- build QT, KT = [64, S] transposed layouts via PE ----
            QT = qt_pool.tile([64, S], FP32, tag="QT")
            KT = qt_pool.tile([64, S], FP32, tag="KT")
            for src, dst, use_act in ((Qn, QT, True), (Kn, KT, False)):
                for g in range(NB // 4):
                    pt = ps_misc.tile([64, 512], FP32, tag="misc")
                    for j in range(4):
                        c = g * 4 + j
                        # exact fp32 transpose as a regular matmul: src.T @ I
                        nc.tensor.matmul(
                            pt[:, j * 128 : (j + 1) * 128],
                            lhsT=src[:, c, :],
                            rhs=ident,
                            start=True,
                            stop=True,
                        )
                    if use_act:
                        nc.scalar.copy(dst[:, g * 512 : (g + 1) * 512], pt)
                    else:
                        nc.vector.tensor_copy(dst[:, g * 512 : (g + 1) * 512], pt)

            for qb in range(NQUARTERS):
                kvmax = (qb * QBLKS + QBLKS) * 128 if causal else S
                kcmax = kvmax // 128  # k-chunks consumed by this quarter
                # A quarter buffer: 4 q-blocks x full key row
                Aq = aq_pool.tile([128, QBLKS, S], FP32, tag="Aq")
                # A^T staging: [k-part, k-chunk, q-within-quarter]
                AT = at_pool.tile([128, NB, 512], F16, tag="AT")

                for i in range(QBLKS):
                    qi = qb * QBLKS + i
                    kv = (qi + 1) * 128 if causal else S

                    if not causal:
                        pent = pen_pool.tile([128, S], FP32, tag="pent")
                        nc.gpsimd.dma_start(
                            out=pent, in_=pen_d[qi * 128 : (qi + 1) * 128, :]
                        )

                    # ---- scores into PSUM, [128,1024] tiles ----
                    stiles = []
                    for c0 in range(0, kv, 1024):
                        w = min(1024, kv - c0)
                        st = ps_scores.tile([128, 1024], FP32, tag="scores")
                        stiles.append((st, c0, w))
                        for n0 in range(0, w, 512):
                            n = min(512, w - n0)
                            nc.tensor.matmul(
                                st[:, n0 : n0 + n],
                                lhsT=QT[:, qi * 128 : (qi + 1) * 128],
                                rhs=KT[:, c0 + n0 : c0 + n0 + n],
                                start=True,
                                stop=True,
                            )
                        if not causal:
                            # add penalty rows into every chunk: I.T @ pen
                            for n0 in range(0, w, 512):
                                n = min(512, w - n0)
                                nc.tensor.matmul(
                                    st[:, n0 : n0 + n],
                                    lhsT=ident,
                                    rhs=pent[:, c0 + n0 : c0 + n0 + n],
                                    start=False,
                                    stop=True,
                                    skip_group_check=True,
                                )
                    if causal:
                        # only the diagonal block needs masking (I.T @ pen
                        # accumulated into PSUM; in-place DVE adds on PSUM
                        # are not safe - single-ported banks)
                        st, c0, _w = stiles[-1]
                        off = qi * 128 - c0
                        nc.tensor.matmul(
                            st[:, off : off + 128],
                            lhsT=ident,
                            rhs=pen[:, qi * 128 : (qi + 1) * 128],
                            start=False,
                            stop=True,
                            skip_group_check=True,
                        )

                    # ---- exp from PSUM with fused row sums ----
                    sums_parts = sums_pool.tile([128, 2], FP32, tag="sums")
                    for t, (st, c0, w) in enumerate(stiles):
                        nc.scalar.activation(
                            out=Aq[:, i, c0 : c0 + w],
                            in_=st[:, 0:w],
                            func=AF.Exp,
                            scale=scale,
                            accum_out=sums_parts[:, t : t + 1],
                        )

                    recip = sums_pool.tile([128, 1], FP32, tag="recip")
                    if len(stiles) > 1:
                        ssum = sums_pool.tile([128, 1], FP32, tag="ssum")
                        nc.vector.tensor_add(
                            ssum, sums_parts[:, 0:1], sums_parts[:, 1:2]
                        )
                        nc.vector.reciprocal(recip, ssum)
                    else:
                        nc.vector.reciprocal(recip, sums_parts[:, 0:1])

                    # ---- fp16 normalized copy for the A@V path (DVE 2x) ----
                    A16 = a16_pool.tile([128, S], F16, tag="A16")
                    nc.vector.tensor_scalar_mul(
                        A16[:, 0:kv], Aq[:, i, 0:kv], recip
                    )

                    # ---- normalize in place (DVE 2x) ----
                    nc.vector.tensor_scalar_mul(
                        Aq[:, i, 0:kv], Aq[:, i, 0:kv], recip
                    )

                    # ---- transpose A row-block into AT staging (bf16) ----
                    nkc = kv // 128
                    for g0 in range(0, nkc, 4):
                        gn = min(4, nkc - g0)
                        pat = ps_at.tile([128, 512], FP32, tag="pat")
                        for j in range(gn):
                            kc = g0 + j
                            nc.tensor.matmul(
                                pat[:, j * 128 : (j + 1) * 128],
                                lhsT=A16[:, kc * 128 : (kc + 1) * 128],
                                rhs=ident16,
                                start=True,
                                stop=True,
                            )
                        dst = AT[:, g0 : g0 + gn, i * 128 : (i + 1) * 128]
                        src = pat[:, 0 : gn * 128].rearrange(
                            "p (g j) -> p g j", g=gn
                        )
                        if (g0 // 4) % 2 == 0:
                            nc.vector.tensor_copy(dst, src)
                        else:
                            nc.scalar.copy(dst, src)
                    if causal:
                        # zero the not-yet-attended staircase inside quarter
                        for kc in range(nkc, kcmax):
                            nc.gpsimd.memset(
                                AT[:, kc, i * 128 : (i + 1) * 128], 0.0
                            )

                if causal:
                    # zero Aq staircase so one rectangular DMA covers quarter
                    for i in range(QBLKS - 1):
                        kv_i = (qb * QBLKS + i + 1) * 128
                        nc.gpsimd.memset(Aq[:, i, kv_i:kvmax], 0.0)

                # ---- attention quarter write ----
                att_dst = att_d[h, qb * 512 : (qb + 1) * 512, 0:kvmax].rearrange(
                    "(c p) k -> p c k", p=128
                )
                nc.sync.dma_start(out=att_dst, in_=Aq[:, :, 0:kvmax])

                # ---- out^T = sum_kc V[kc]-stationary @ A^T[kc] ----
                poT = ps_ot.tile([64, 512], FP32, tag="poT")
                for kc in range(kcmax):
                    nc.tensor.matmul(
                        poT,
                        lhsT=V16[:, kc, :],
                        rhs=AT[:, kc, :],
                        start=(kc == 0),
                        stop=(kc == kcmax - 1),
                    )
                oT = o_pool.tile([64, 512], FP32, tag="oT")
                nc.scalar.copy(oT, poT)
                pout = ps_misc.tile([128, 256], FP32, tag="misc")
                for j in range(4):
                    nc.tensor.matmul(
                        pout[:, j * 64 : (j + 1) * 64],
                        lhsT=oT[:, j * 128 : (j + 1) * 128],
                        rhs=ident[0:64, 0:64],
                        start=True,
                        stop=True,
                    )
                ob = o_pool.tile([128, 4, 64], FP32, tag="ob")
                nc.vector.tensor_copy(
                    ob, pout.rearrange("p (c d) -> p c d", c=4)
                )
                out_dst = out_d[h, qb * 512 : (qb + 1) * 512, :].rearrange(
                    "(c p) d -> p c d", p=128
                )
                nc.sync.dma_start(out=out_dst, in_=ob)


def build_program(causal: bool, scale: float):
    nc = bacc.Bacc(
        "TRN2",
        target_bir_lowering=False,
        debug=False,
        enable_asserts=False,
        num_devices=N_CORES,
    )
    hp = HEADS_PER_CORE
    ins = {
        "q": nc.dram_tensor("q", [hp, S, D], FP32, kind="ExternalInput").ap(),
        "k": nc.dram_tensor("k", [hp, S, D], FP32, kind="ExternalInput").ap(),
        "v": nc.dram_tensor("v", [hp, S, D], FP32, kind="ExternalInput").ap(),
        "ident": nc.dram_tensor(
            "ident", [128, 128], FP32, kind="ExternalInput"
        ).ap(),
    }
    pen_shape = [128, S] if causal else [S, S]
    ins["pen"] = nc.dram_tensor(
        "pen", pen_shape, FP32, kind="ExternalInput"
    ).ap()
    outs = {
        "att": nc.dram_tensor(
            "att", [hp, S, S], FP32, kind="ExternalOutput"
        ).ap(),
        "out": nc.dram_tensor(
            "out", [hp, S, D], FP32, kind="ExternalOutput"
        ).ap(),
    }
    with tile.TileContext(nc) as tc:
        _attention_body(tc, outs, ins, causal=causal, scale=scale)
    nc.compile()
    return nc


_PROGRAM_CACHE = {}


def _get_program(causal: bool, scale: float):
    key = (causal, scale)
    if key not in _PROGRAM_CACHE:
        _PROGRAM_CACHE[key] = build_program(causal, scale)
    return _PROGRAM_CACHE[key]


def kernel(q, k, v, mask, d_key, mask_value):
    q = np.ascontiguousarray(np.asarray(q, dtype=np.float32))
    k = np.ascontiguousarray(np.asarray(k, dtype=np.float32))
    v = np.ascontiguousarray(np.asarray(v, dtype=np.float32))
    mask2d = np.asarray(mask).reshape(S, S)
    scale = 1.0 / math.sqrt(float(np.asarray(d_key)))

    causal = bool(
        np.array_equal(mask2d != 0, np.tril(np.ones((S, S), dtype=bool)))
    )

    maskf = (mask2d != 0).astype(np.float32)
    if causal:
        # per-q-block diagonal penalty blocks, stacked along free dim
        pen = np.zeros((128, S), dtype=np.float32)
        for qi in range(NB):
            blk = maskf[qi * 128 : (qi + 1) * 128, qi * 128 : (qi + 1) * 128]
            pen[:, qi * 128 : (qi + 1) * 128] = (1.0 - blk) * PENALTY
    else:
        pen = (1.0 - maskf) * PENALTY

    ident = np.eye(128, dtype=np.float32)

    qr = q.reshape(B * H, S, D)
    kr = k.reshape(B * H, S, D)
    vr = v.reshape(B * H, S, D)

    nc = _get_program(causal, scale)
    in_maps = []
    for c in range(N_CORES):
        sl = slice(c * HEADS_PER_CORE, (c + 1) * HEADS_PER_CORE)
        in_maps.append(
            {
                "q": np.ascontiguousarray(qr[sl]),
                "k": np.ascontiguousarray(kr[sl]),
                "v": np.ascontiguousarray(vr[sl]),
                "pen": pen,
                "ident": ident,
            }
        )

    trace = os.environ.get("KERNEL_TRACE") == "1"
    res = run_bass_kernel_spmd(
        nc, in_maps, core_ids=list(range(N_CORES)), trace=trace
    )
    if trace:
        print(f"HW exec time: {res.exec_time_ns} ns")

    att = np.empty((B * H, S, S), dtype=np.float32)
    out = np.empty((B * H, S, D), dtype=np.float32)
    for c in range(N_CORES):
        sl = slice(c * HEADS_PER_CORE, (c + 1) * HEADS_PER_CORE)
        att[sl] = res.results[c]["att"]
        out[sl] = res.results[c]["out"]
    return out.reshape(B, H, S, D), att.reshape(B, H, S, S)
